# revision 1
# baseline (speedup 1.0000x reference)
"""EntropyByteLatentTransformer on 8 Trainium2 NeuronCores.

Strategy: pure data-parallel over batch B=8 (one sequence per core).
Per core: full 8-layer transformer (D=512, H=8, L=1024, DFF=2048) plus
sliding-window byte-entropy features, embedding via one-hot matmul, and
the V=258 unembedding.

Layouts: residual h is token-major [l(part), d(free)]; matmul operands are
produced feature-major via PE transposes; attention probabilities are
softmaxed in [l, m] layout and moved to [m, l] with the DMA xbar transpose
(fp16).  Matmuls run as fp32r (full PE rate at N>=256) or fp16.
"""

import numpy as np

import concourse.bacc as bacc
import concourse.mybir as mybir
import concourse.tile as tile
from concourse.bass_utils import run_bass_kernel_spmd

F32 = mybir.dt.float32
F32R = mybir.dt.float32r
F16 = mybir.dt.float16
AF = mybir.ActivationFunctionType
ALU = mybir.AluOpType
AX = mybir.AxisListType

D = 512
H = 8
NL = 8
V = 258
W = 8
DFF = 2048
DH = D // H
L = 1024
B = 8
NCORES = 8

LC = L // 128      # 8 l-chunks of 128 tokens
DC = D // 128      # 4 d-chunks
JQK = 2 * D // 128  # 8 q+k feature chunks
JF = DFF // 128    # 16 dff chunks
NW = L - W + 1     # 1017 valid entropy windows


def _build_nc(n_layers=NL, dbg=False):
    nc = bacc.Bacc("TRN2", target_bir_lowering=False, debug=False,
                   num_devices=NCORES)

    dt_in = {}

    def din(name, shape, dtype):
        dt_in[name] = nc.dram_tensor(name, list(shape), dtype,
                                     kind="ExternalInput")
        return dt_in[name]

    xf = din("xf", [1, L], F32R)
    iota = din("iota", [128, 2], F32)
    ones1 = din("ones1", [1, 128], F32R)
    idn = din("idn", [128, 128], F32)
    emb = din("emb", [128, 2, D], F32R)
    went = din("went", [128, D], F32R)
    wout = din("wout", [128, DC, V], F16)
    wqk = din("wqk", [NL, 128, DC, JQK, 128], F16)
    bqk = din("bqk", [NL, 128, JQK], F32)
    wv = din("wv", [NL, 128, DC, D], F16)
    bvb = din("bvb", [NL, 128, D], F32)
    wop = din("wop", [NL, 128, DC, DC, 128], F16)
    bop = din("bop", [NL, 128, DC], F32)
    wf1 = din("wf1", [NL, JF, 128, DC, 128], F16)
    bf1 = din("bf1", [NL, 128, JF], F32)
    wf2 = din("wf2", [NL, JF, 128, DC, 128], F16)
    bf2 = din("bf2", [NL, 128, DC], F32)

    out_d = nc.dram_tensor("out", [L, V], F32, kind="ExternalOutput")
    dbgt = {}
    if dbg:
        for nm, shp, dt in [("dbg_h0", [128, LC, D], F32),
                        ("dbg_g", [128, LC, D], F32),
                        ("dbg_gfm", [128, DC, L], F32),
                        ("dbg_qk", [128, JQK, L], F16),
                        ("dbg_v", [128, LC, D], F16), ("dbg_es", [128, L], F16),
                        ("dbg_pt", [128, 8, 512], F16),
                        ("dbg_o", [128, DC, L], F16),
                        ("dbg_hp", [128, DC, L], F32),
                        ("dbg_h1", [128, LC, D], F32)]:
            dbgt[nm] = nc.dram_tensor(nm, shp, dt, kind="ExternalOutput")

    with tile.TileContext(nc) as tc:
        with tc.tile_pool(name="consts", bufs=1) as cp, \
             tc.tile_pool(name="persist", bufs=1) as pp, \
             tc.tile_pool(name="wpool", bufs=1) as wp, \
             tc.tile_pool(name="wstream", bufs=3) as ws, \
             tc.tile_pool(name="work", bufs=1) as wk, \
             tc.tile_pool(name="psum", bufs=8, space="PSUM") as ps:

            _ctr = [0]

            def pst():
                _ctr[0] += 1
                return ps.tile([128, 512], F32, tag="p",
                               name=f"ps{_ctr[0]}")

            # ---- constants ----
            ident = cp.tile([128, 128], F32)
            nc.sync.dma_start(ident[:], idn.ap())
            iota_sb = cp.tile([128, 2], F32)
            nc.sync.dma_start(iota_sb[:], iota.ap())
            ones_sb = cp.tile([1, 128], F32R)
            nc.sync.dma_start(ones_sb[:], ones1.ap())
            emb_sb = cp.tile([128, 2, D], F32R)
            nc.sync.dma_start(emb_sb[:], emb.ap())
            went_sb = cp.tile([128, D], F32R)
            nc.sync.dma_start(went_sb[:], went.ap())
            wout_sb = cp.tile([128, DC, V], F16)
            nc.sync.dma_start(wout_sb[:], wout.ap())
            eps_sb = cp.tile([128, 1], F32)
            nc.vector.memset(eps_sb[:], 1e-5)

            # ---- persistent activations ----
            h_all = pp.tile([128, LC, D], F32)          # residual, token-major
            g_all = pp.tile([128, LC, D], F32)          # LN output, token-major
            g_fm = pp.tile([128, DC, L], F16)          # LN output, feature-major
            qk_fm = pp.tile([128, JQK, L], F16)         # Q,K feature-major
            v_tm = pp.tile([128, LC, D], F16)           # V token-major
            o_fm = pp.tile([128, DC, L], F16)           # attn out, feature-major
            hp_fm = pp.tile([128, DC, L], F32)          # block out, feature-major

            # ---- per-layer weights (single-buffered, reloaded per layer) ----
            wqk_sb = wp.tile([128, DC, JQK, 128], F16, tag="wqk")
            bqk_sb = wp.tile([128, JQK], F32, tag="bqk")
            wv_sb = wp.tile([128, DC, D], F16, tag="wv")
            bvb_sb = wp.tile([128, D], F32, tag="bvb")
            wop_sb = wp.tile([128, DC, DC, 128], F16, tag="wop")
            bop_sb = wp.tile([128, DC], F32, tag="bop")
            wf1_sb = wp.tile([128, JF, DC, 128], F16, tag="wf1r")
            bf1_sb = wp.tile([128, JF], F32, tag="bf1")
            bf2_sb = wp.tile([128, DC], F32, tag="bf2")

            # =========================================================
            # Embedding + entropy features
            # =========================================================
            with tc.tile_pool(name="embed", bufs=1) as ep:
                x1 = ep.tile([1, L], F32R)
                nc.sync.dma_start(x1[:], xf.ap())
                # embed-phase scratch aliases persistent tiles not yet in use
                xb = hp_fm[:, 0, :]
                p_t = hp_fm[:, 1, :]
                lnp_t = hp_fm[:, 2, :]
                oh = [ep.tile([128, L], F32R, name=f"oh{i}") for i in range(2)]
                plog = [ep.tile([128, L], F32R, name=f"plog{i}")
                        for i in range(2)]
                cc = [qk_fm[:, 0, :], qk_fm[:, 1, :], qk_fm[:, 2, :]]
                for mh in range(2):
                    pb = pst()
                    nc.tensor.matmul(pb[:], ones_sb[:],
                                     x1[:, mh * 512:(mh + 1) * 512],
                                     start=True, stop=True)
                    nc.vector.tensor_copy(xb[:, mh * 512:(mh + 1) * 512], pb[:])

                for half in range(2):
                    nc.vector.tensor_scalar(
                        oh[half][:], xb[:], iota_sb[:, half:half + 1], None,
                        op0=ALU.is_equal)
                    c2, c4, c8 = cc
                    nc.vector.tensor_tensor(
                        c2[:, 0:L - 1], oh[half][:, 0:L - 1],
                        oh[half][:, 1:L], op=ALU.add)
                    nc.vector.tensor_tensor(
                        c4[:, 0:L - 3], c2[:, 0:L - 3], c2[:, 2:L - 1],
                        op=ALU.add)
                    nc.vector.tensor_tensor(
                        c8[:, 0:NW], c4[:, 0:NW], c4[:, 4:NW + 4], op=ALU.add)
                    # p = max(c/8, 1e-10); plog = p * ln(p), zero-padded
                    nc.vector.tensor_scalar(
                        p_t[:, 0:NW], c8[:, 0:NW], 0.125, 1e-10,
                        op0=ALU.mult, op1=ALU.max)
                    nc.scalar.activation(lnp_t[:, 0:NW], p_t[:, 0:NW], AF.Ln)
                    nc.vector.memset(plog[half][:].bitcast(mybir.dt.uint32), 0)
                    nc.vector.tensor_tensor(
                        plog[half][:, 0:NW], p_t[:, 0:NW], lnp_t[:, 0:NW],
                        op=ALU.mult)

                for lc in range(LC):
                    sl = slice(lc * 128, (lc + 1) * 128)
                    ph = pst()
                    nc.tensor.matmul(ph[:], oh[0][:, sl], emb_sb[:, 0, :],
                                     start=True, stop=False)
                    nc.tensor.matmul(ph[:], oh[1][:, sl], emb_sb[:, 1, :],
                                     start=False, stop=False)
                    nc.tensor.matmul(ph[:], plog[0][:, sl], went_sb[:],
                                     start=False, stop=False)
                    nc.tensor.matmul(ph[:], plog[1][:, sl], went_sb[:],
                                     start=False, stop=True)
                    nc.vector.tensor_copy(h_all[:, lc, :], ph[:])
                if dbg:
                    nc.sync.dma_start(dbgt["dbg_h0"].ap(), h_all[:])

            # =========================================================
            # Transformer layers
            # =========================================================
            stats = wk.tile([128, 8 * LC], F32, tag="stats")

            def layer_norm(li, ln_idx, resid_src=None):
                """[h_all += T(resid_src)] -> g_all -> g_fm, pipelined per lc."""
                for lc in range(LC):
                    if resid_src is not None:
                        pt = pst()
                        for dc in range(DC):
                            nc.tensor.transpose(
                                pt[:, dc * 128:(dc + 1) * 128],
                                resid_src[:, dc, lc * 128:(lc + 1) * 128],
                                ident[:])
                        nc.vector.tensor_tensor(
                            h_all[:, lc, :].rearrange("p (c f) -> p c f", c=DC),
                            pt[:].rearrange("p (c f) -> p c f", c=DC),
                            h_all[:, lc, :].rearrange("p (c f) -> p c f", c=DC),
                            op=ALU.add)
                    st = stats[:, 8 * lc:8 * lc + 8]
                    sq = wk.tile([128, D], F32, tag="sq", bufs=2,
                                 name=f"sq{li}_{ln_idx}_{lc}")
                    nc.scalar.activation(sq[:], h_all[:, lc, :], AF.Square,
                                         accum_out=st[:, 0:1])
                    nc.vector.reduce_sum(st[:, 1:2], h_all[:, lc, :], axis=AX.X)
                    nc.vector.tensor_scalar(st[:, 2:3], st[:, 1:2], 1.0 / D,
                                            None, op0=ALU.mult)  # mu
                    nc.vector.tensor_tensor(st[:, 3:4], st[:, 2:3], st[:, 2:3],
                                            op=ALU.mult)  # mu^2
                    nc.vector.scalar_tensor_tensor(
                        st[:, 4:5], st[:, 0:1], 1.0 / D, st[:, 3:4],
                        op0=ALU.mult, op1=ALU.subtract)  # var
                    nc.scalar.activation(st[:, 6:7], st[:, 4:5], AF.Sqrt,
                                         bias=eps_sb[:])
                    nc.vector.reciprocal(st[:, 7:8], st[:, 6:7])
                    nc.vector.tensor_scalar(
                        g_all[:, lc, :], h_all[:, lc, :], st[:, 2:3],
                        st[:, 7:8], op0=ALU.subtract, op1=ALU.mult)
                    pt2 = pst()
                    for dc in range(DC):
                        nc.tensor.transpose(
                            pt2[:, dc * 128:(dc + 1) * 128],
                            g_all[:, lc, dc * 128:(dc + 1) * 128], ident[:])
                    nc.vector.tensor_copy(
                        g_fm[:, :, lc * 128:(lc + 1) * 128],
                        pt2[:].rearrange("p (c f) -> p c f", c=DC))

            def feature_to_resid(src_fm):
                """h_all += transpose(src_fm)."""
                for lc in range(LC):
                    pt = pst()
                    for dc in range(DC):
                        nc.tensor.transpose(
                            pt[:, dc * 128:(dc + 1) * 128],
                            src_fm[:, dc, lc * 128:(lc + 1) * 128], ident[:])
                    nc.vector.tensor_tensor(
                        h_all[:, lc, :].rearrange("p (c f) -> p c f", c=DC),
                        pt[:].rearrange("p (c f) -> p c f", c=DC),
                        h_all[:, lc, :].rearrange("p (c f) -> p c f", c=DC),
                        op=ALU.add)

            for li in range(n_layers):
                # ---- weight loads for this layer ----
                nc.sync.dma_start(wqk_sb[:], wqk.ap()[li])
                nc.sync.dma_start(bqk_sb[:], bqk.ap()[li])
                nc.sync.dma_start(wv_sb[:], wv.ap()[li])
                nc.sync.dma_start(bvb_sb[:], bvb.ap()[li])
                nc.sync.dma_start(wop_sb[:], wop.ap()[li])
                nc.sync.dma_start(bop_sb[:], bop.ap()[li])
                nc.sync.dma_start(
                    wf1_sb[:], wf1.ap()[li].rearrange("j p c f -> p j c f"))
                nc.sync.dma_start(bf1_sb[:], bf1.ap()[li])
                nc.sync.dma_start(bf2_sb[:], bf2.ap()[li])

                layer_norm(li, 0, resid_src=(hp_fm if li > 0 else None))
                if dbg and li == 0:
                    nc.sync.dma_start(dbgt["dbg_g"].ap(), g_all[:])
                    nc.sync.dma_start(dbgt["dbg_gfm"].ap(),
                                      g_fm[:].bitcast(F32))

                # ---- QKV (feature-major Q,K) ----
                for j in range(JQK):
                    for lh in range(2):
                        pq = pst()
                        for dc in range(DC):
                            nc.tensor.matmul(
                                pq[:], wqk_sb[:, dc, j, :],
                                g_fm[:, dc, lh * 512:(lh + 1) * 512],
                                start=(dc == 0), stop=(dc == DC - 1))
                        nc.vector.tensor_scalar(
                            qk_fm[:, j, lh * 512:(lh + 1) * 512], pq[:],
                            bqk_sb[:, j:j + 1], None, op0=ALU.add)

                # ---- V (token-major) ----
                for lc in range(LC):
                    pv = pst()
                    for dc in range(DC):
                        nc.tensor.matmul(
                            pv[:], g_fm[:, dc, lc * 128:(lc + 1) * 128],
                            wv_sb[:, dc, :],
                            start=(dc == 0), stop=(dc == DC - 1))
                    nc.vector.tensor_tensor(
                        v_tm[:, lc, :], pv[:], bvb_sb[:], op=ALU.add)

                if dbg and li == 0:
                    nc.sync.dma_start(dbgt["dbg_qk"].ap(), qk_fm[:])
                    nc.sync.dma_start(dbgt["dbg_v"].ap(), v_tm[:])

                # ---- attention, head pair by head pair ----
                for p in range(4):
                    hA, hB = 2 * p, 2 * p + 1
                    for lh in range(2):
                        ptr = [
                            wk.tile([128, 8, 512], F16, tag="PT0", bufs=2,
                                    name=f"pt0_{li}_{p}_{lh}"),
                            wk.tile([128, 8, 512], F16, tag="PT1", bufs=2,
                                    name=f"pt1_{li}_{p}_{lh}"),
                        ]
                        for lcs in range(4):
                            lc = lh * 4 + lcs
                            lsl = slice(lc * 128, (lc + 1) * 128)
                            for hi, (p0, p1, tp) in enumerate(
                                    ((0, 64, (0, 0)), (64, 128, (64, 0)))):
                                smm = [pst(), pst()]
                                for mh in range(2):
                                    msl = slice(mh * 512, (mh + 1) * 512)
                                    nc.tensor.matmul(
                                        smm[mh][:],
                                        qk_fm[p0:p1, p, lsl],
                                        qk_fm[p0:p1, 4 + p, msl],
                                        start=True, stop=True,
                                        tile_position=tp)
                                es = wk.tile([128, L], F16, tag="eS", bufs=3)
                                sacc = wk.tile([128, 4], F32, tag="sacc",
                                               bufs=8)
                                for mh in range(2):
                                    nc.scalar.activation(
                                        es[:, mh * 512:(mh + 1) * 512],
                                        smm[mh][:], AF.Exp,
                                        accum_out=sacc[:, mh:mh + 1])
                                nc.vector.tensor_tensor(
                                    sacc[:, 2:3], sacc[:, 0:1], sacc[:, 1:2],
                                    op=ALU.add)
                                nc.vector.reciprocal(sacc[:, 3:4], sacc[:, 2:3])
                                nc.vector.tensor_scalar(
                                    es[:], es[:], sacc[:, 3:4], None,
                                    op0=ALU.mult)
                                nc.sync.dma_start_transpose(
                                    ptr[hi][:, :, lcs * 128:(lcs + 1) * 128],
                                    es[:])
                                if dbg and li == 0 and p == 0 and lh == 0 \
                                        and lcs == 0 and hi == 0:
                                    nc.sync.dma_start(dbgt["dbg_es"].ap(),
                                                      es[:])
                        # attn @ V for this l-half (one psum chain per head;
                        # packed col-group accumulation is order-sensitive on
                        # HW and the scheduler may reorder across heads)
                        for hi, hh in enumerate((hA, hB)):
                            po = pst()
                            for mc in range(LC):
                                nc.tensor.matmul(
                                    po[0:64, :],
                                    v_tm[:, mc, hh * 64:(hh + 1) * 64],
                                    ptr[hi][:, mc, :],
                                    start=(mc == 0), stop=(mc == LC - 1))
                            nc.vector.tensor_copy(
                                o_fm[hi * 64:(hi + 1) * 64, p,
                                     lh * 512:(lh + 1) * 512], po[0:64, :])
                        if dbg and li == 0 and p == 0 and lh == 0:
                            nc.sync.dma_start(dbgt["dbg_pt"].ap(), ptr[0][:])

                # ---- output projection ----
                for m in range(DC):
                    for lh in range(2):
                        pj = pst()
                        for dc in range(DC):
                            nc.tensor.matmul(
                                pj[:], wop_sb[:, dc, m, :],
                                o_fm[:, dc, lh * 512:(lh + 1) * 512],
                                start=(dc == 0), stop=(dc == DC - 1))
                        nc.vector.tensor_scalar(
                            hp_fm[:, m, lh * 512:(lh + 1) * 512], pj[:],
                            bop_sb[:, m:m + 1], None, op0=ALU.add)
                if dbg and li == 0:
                    nc.sync.dma_start(dbgt["dbg_o"].ap(), o_fm[:])
                    nc.sync.dma_start(dbgt["dbg_hp"].ap(), hp_fm[:])
                feature_to_resid(hp_fm)
                if dbg and li == 0:
                    nc.sync.dma_start(dbgt["dbg_h1"].ap(), h_all[:])

                layer_norm(li, 1)

                # ---- FFN ----
                for lh in range(2):
                    lsl = slice(lh * 512, (lh + 1) * 512)
                    pf2 = [pst() for _ in range(DC)]
                    for j in range(JF):
                        w2j = ws.tile([128, DC, 128], F16, tag="wf2")
                        nc.sync.dma_start(w2j[:], wf2.ap()[li, j])
                        pf1 = pst()
                        for dc in range(DC):
                            nc.tensor.matmul(
                                pf1[:], wf1_sb[:, j, dc, :], g_fm[:, dc, lsl],
                                start=(dc == 0), stop=(dc == DC - 1))
                        u = wk.tile([128, 512], F16, tag="u", bufs=3)
                        nc.scalar.activation(u[:], pf1[:], AF.Gelu,
                                             bias=bf1_sb[:, j:j + 1])
                        for m in range(DC):
                            nc.tensor.matmul(
                                pf2[m][:], w2j[:, m, :], u[:],
                                start=(j == 0), stop=(j == JF - 1))
                    for m in range(DC):
                        nc.vector.tensor_scalar(
                            hp_fm[:, m, lsl], pf2[m][:],
                            bf2_sb[:, m:m + 1], None, op0=ALU.add)
                if li == NL - 1:
                    feature_to_resid(hp_fm)

            # =========================================================
            # Unembedding (reuses g_fm as transposed-h buffer)
            # =========================================================
            for lc in range(LC):
                pt = pst()
                for dc in range(DC):
                    nc.tensor.transpose(
                        pt[:, dc * 128:(dc + 1) * 128],
                        h_all[:, lc, dc * 128:(dc + 1) * 128], ident[:])
                nc.vector.tensor_copy(
                    g_fm[:, :, lc * 128:(lc + 1) * 128],
                    pt[:].rearrange("p (c f) -> p c f", c=DC))
            for lc in range(LC):
                pu = ps.tile([128, V], F32, tag="p", name=f"pu{lc}")
                for dc in range(DC):
                    nc.tensor.matmul(
                        pu[:], g_fm[:, dc, lc * 128:(lc + 1) * 128],
                        wout_sb[:, dc, :],
                        start=(dc == 0), stop=(dc == DC - 1))
                osb = wk.tile([128, V], F32, tag="osb", bufs=2)
                nc.vector.tensor_copy(osb[:], pu[:])
                nc.sync.dma_start(out_d.ap()[lc * 128:(lc + 1) * 128, :],
                                  osb[:])

    nc.finalize()
    return nc


def _prep_weights(inp):
    """Host-side weight layout transforms (fp32 numpy)."""
    f = lambda k: np.asarray(inp[k], np.float32)
    embed, ep_w, ep_b = f("embed"), f("ep_w"), f("ep_b")
    in_w, in_b = f("in_w"), f("in_b")
    op_w, op_b = f("op_w"), f("op_b")
    ln1_w, ln1_b = f("ln1_w"), f("ln1_b")
    ln2_w, ln2_b = f("ln2_w"), f("ln2_b")
    f1_w, f1_b = f("f1_w"), f("f1_b")
    f2_w, f2_b = f("f2_w"), f("f2_b")
    out_w = f("out_w")

    m = {}
    m["iota"] = np.stack([np.arange(128), np.arange(128, 256)],
                         axis=1).astype(np.float32)
    m["ones1"] = np.ones((1, 128), np.float32)
    m["idn"] = np.eye(128, dtype=np.float32)
    emb_eff = embed[:256] + ep_b[None, :]
    m["emb"] = np.ascontiguousarray(
        emb_eff.reshape(2, 128, D).transpose(1, 0, 2))
    m["went"] = np.tile((-ep_w[:, 0] / np.log(2.0))[None, :], (128, 1)) \
        .astype(np.float32)
    m["wout"] = np.ascontiguousarray(
        out_w.T.reshape(DC, 128, V).transpose(1, 0, 2)).astype(np.float16)

    wqk = np.empty((NL, 128, DC, JQK, 128), np.float32)
    bqk = np.empty((NL, 128, JQK), np.float32)
    wv = np.empty((NL, 128, DC, D), np.float32)
    bvb = np.empty((NL, 128, D), np.float32)
    wop = np.empty((NL, 128, DC, DC, 128), np.float32)
    bop = np.empty((NL, 128, DC), np.float32)
    wf1 = np.empty((NL, JF, 128, DC, 128), np.float32)
    bf1 = np.empty((NL, 128, JF), np.float32)
    wf2 = np.empty((NL, JF, 128, DC, 128), np.float32)
    bf2 = np.empty((NL, 128, DC), np.float32)
    for li in range(NL):
        Weff = in_w[li] * ln1_w[li][None, :]
        beff = in_w[li] @ ln1_b[li] + in_b[li]
        Weff = Weff.copy()
        beff = beff.copy()
        Weff[:D] *= 1.0 / np.sqrt(DH)     # fold 1/sqrt(dh) into q
        beff[:D] *= 1.0 / np.sqrt(DH)
        WqkT = Weff[:2 * D].T             # [512, 1024]
        wqk[li] = WqkT.reshape(DC, 128, JQK, 128).transpose(1, 0, 2, 3)
        bqk[li] = beff[:2 * D].reshape(JQK, 128).T
        WvT = Weff[2 * D:].T              # [512, 512]
        wv[li] = WvT.reshape(DC, 128, D).transpose(1, 0, 2)
        bvb[li] = np.tile(beff[2 * D:][None, :], (128, 1))
        wop[li] = op_w[li].T.reshape(DC, 128, DC, 128).transpose(1, 0, 2, 3)
        bop[li] = op_b[li].reshape(DC, 128).T
        F1eff = f1_w[li] * ln2_w[li][None, :]
        b1eff = f1_w[li] @ ln2_b[li] + f1_b[li]
        wf1[li] = F1eff.T.reshape(DC, 128, JF, 128).transpose(2, 1, 0, 3)
        bf1[li] = b1eff.reshape(JF, 128).T
        wf2[li] = f2_w[li].T.reshape(JF, 128, DC, 128)
        bf2[li] = f2_b[li].reshape(DC, 128).T
    m.update(wqk=wqk.astype(np.float16), bqk=bqk, wv=wv.astype(np.float16),
             bvb=bvb, wop=wop, bop=bop, wf1=wf1.astype(np.float16), bf1=bf1,
             wf2=np.asarray(wf2), bf2=bf2)
    m["wop"] = m["wop"].astype(np.float16)
    m["wf2"] = m["wf2"].astype(np.float16)
    return m


_NC_CACHE = {}


def kernel(**inputs):
    x = np.asarray(inputs["x"])
    assert x.shape == (B, L)
    if "nc" not in _NC_CACHE:
        _NC_CACHE["nc"] = _build_nc()
    nc = _NC_CACHE["nc"]
    w = _prep_weights(inputs)
    in_maps = []
    for c in range(B):
        mm = dict(w)
        mm["xf"] = x[c].astype(np.float32)[None, :]
        in_maps.append(mm)
    res = run_bass_kernel_spmd(nc, in_maps, core_ids=list(range(NCORES)))
    out = np.stack([res.results[c]["out"] for c in range(B)], axis=0)
    return out.astype(np.float32)

